# revision 24
# baseline (speedup 1.0000x reference)
"""Trainium2 Bass kernel for nn_MemoryAggregator — ap_gather edition.

Reference computation:
    Q = X@Wq; K = X@Wk; V = X@Wv            (X [100000,256], W [256,32])
    scores_e = <Q[src_e], K[dst_e]> / sqrt(32)   over 1.6M edges
    out[n]   = softmax-weighted sum over n's edges of V[dst_e]   ([100000,32])

Strategy (8 NeuronCores, SPMD, edges sharded by src):
  kernel1: per-core QKV projections (PE matmul), as before.
  kernel2: per core, per-edge pipeline driven by GPSIMD ap_gather:
    - K|V table [128 chan, 12500, 4] bf16 resident in SBUF: channel 16k+c
      (dst-chunk k of 12500 nodes, word c) holds [K[2c],K[2c+1],V[2c],V[2c+1]]
      of each chunk node. Each of the 8 GPSIMD CPUs gathers its own chunk's
      edge stream (d=4, ~26.5 ns/idx fixed cost, streams run in parallel).
    - DVE: prod = Qexp * Kgathered (bf16), pair-reduce -> [128, TI] f32.
    - PE: per (chunk, 512-col quarter) ones[16,16]-matmul reduces the 16
      word-partitions -> scores replicated across the chunk's 16 partitions.
    - ACT: exp(score/sqrt(32)) (no max subtraction; scores are O(10), safe).
    - DVE: exv = ex * Vgathered (bf16) -> per-edge partials to HBM.
  host:    segment sums per src (bincount) + divide, as in the baseline.

Softmax max-subtraction is dropped: scores ~ N(0,4), exp safe in f32.
"""
import math
from contextlib import ExitStack

import numpy as np
import ml_dtypes

import concourse.bass as bass
import concourse.tile as tile
from concourse import bacc, mybir
from concourse.bass_utils import run_bass_kernel_spmd

# ---------------------------------------------------------------- dimensions
N = 100000
E = 1600000
D_IN = 256
H = 32
DK = math.sqrt(H)
NCORES = 8
NPC = N // NCORES          # 12500 nodes per core (src shard)
NCHUNK = 8                 # dst chunks, one per GPSIMD CPU group
CHUNK = N // NCHUNK        # 12500
P = 128
TI = 2048                  # edges per chunk-stream per ap_gather call
QUART = 512                # PSUM bank col width (f32)

BF16 = ml_dtypes.bfloat16

_cache = {}
LAST_TIMES = {}


# ================================================================ host prep
def _prep(edge_index):
    """Per-core, per-chunk edge streams (sorted by src within chunk).

    Nodes are assigned to the 8 dst-chunks per core by degree-aware snake
    packing so the 8 gather streams (whose max sets the gather count) are
    balanced to within a few edges instead of the ~±3 sigma of a fixed
    dst//12500 split. node_of[k, l] = global node at (chunk k, local l)."""
    src = np.asarray(edge_index[0], dtype=np.int64)
    dst = np.asarray(edge_index[1], dtype=np.int64)
    core = src // NPC
    cores = []
    node_maps = []
    max_len = 0
    rows = np.arange(N) // NCHUNK
    colp = np.arange(N) % NCHUNK
    snake_chunk = np.where(rows % 2 == 0, colp, NCHUNK - 1 - colp)
    for c in range(NCORES):
        m = core == c
        s_l = src[m] - c * NPC
        d = dst[m]
        cnt = np.bincount(d, minlength=N)
        order = np.argsort(-cnt, kind="stable")
        chunk_of = np.empty(N, dtype=np.int64)
        local_of = np.empty(N, dtype=np.int64)
        chunk_of[order] = snake_chunk
        local_of[order] = rows
        node_of = np.empty((NCHUNK, CHUNK), dtype=np.int64)
        node_of[snake_chunk, rows] = order
        chunk = chunk_of[d]
        dl_all = local_of[d]
        o = np.lexsort((s_l, chunk))
        s_l, dl_all, chunk = s_l[o], dl_all[o], chunk[o]
        bounds = np.searchsorted(chunk, np.arange(NCHUNK + 1))
        streams = []
        for k in range(NCHUNK):
            lo, hi = bounds[k], bounds[k + 1]
            streams.append((s_l[lo:hi], dl_all[lo:hi]))
            max_len = max(max_len, hi - lo)
        cores.append(streams)
        node_maps.append(node_of)
    nt = (max_len + TI - 1) // TI
    tail = max_len - (nt - 1) * TI
    tail = ((tail + 15) // 16) * 16  # num_idxs multiple of 16
    return cores, node_maps, nt, tail


def _pack_core_inputs(streams, nt, tail, Qb, kvt):
    """Build idx / qexp tensors for one core."""
    S = (nt - 1) * TI + tail
    idx = np.zeros((P, S // 16), dtype=np.int16)
    qexp = np.zeros((P, S, 2), dtype=BF16)
    for k in range(NCHUNK):
        sl, dl = streams[k]
        L = len(sl)
        idx_k = np.zeros(S, dtype=np.int16)
        idx_k[:L] = dl.astype(np.int16)
        idx[16 * k : 16 * k + 16, :] = idx_k.reshape(-1, 16).T
        # qexp[16k+c, j, h] = Q[src_j, 2c+h]
        qb = Qb[sl]                      # [L, 32] bf16
        qexp[16 * k : 16 * k + 16, :L, :] = (
            qb.reshape(L, 16, 2).transpose(1, 0, 2)
        )
    return {"kvt": kvt, "idx": idx, "qexp": qexp}


def _combine_core(streams, exd, exvd, nt, tail):
    """Host segment sums + divide for one core. exd [nt,128,TI] f32,
    exvd [nt,128,TI,2] bf16 (last call only :tail valid)."""
    num = np.zeros((NPC, H), dtype=np.float64)
    den = np.zeros(NPC, dtype=np.float64)
    widths = [TI] * (nt - 1) + [tail]
    ex_flat = np.concatenate(
        [exd[i, :, : widths[i]] for i in range(nt)], axis=1
    )                                                          # [128, S]
    exv_flat = np.concatenate(
        [exvd[i, :, : widths[i], :].astype(np.float32) for i in range(nt)], axis=1
    )
    for k in range(NCHUNK):
        sl, _ = streams[k]
        L = len(sl)
        if L == 0:
            continue
        ex_k = ex_flat[16 * k, :L].astype(np.float64)          # [L]
        den += np.bincount(sl, weights=ex_k, minlength=NPC)
        # feats: exv_flat[16k+c, j, h] = ex*V[2c+h]
        blk = exv_flat[16 * k : 16 * k + 16, :L, :]            # [16, L, 2]
        feats = blk.transpose(1, 0, 2).reshape(L, H)           # [L, 32]
        for f in range(H):
            num[:, f] += np.bincount(sl, weights=feats[:, f], minlength=NPC)
    den = np.where(den == 0, 1.0, den)
    return (num / den[:, None]).astype(np.float32)


# ================================================================ kernel 1
K1_COLS = 2048


def _build_k1():
    """Weights-stationary bf16: out qkvT[96, NPC] = W.T @ X.T. Large column
    tiles keep the DMA-instruction count low (issue cost ~0.6 us each);
    matmuls split per 512 cols (one PSUM bank)."""
    nc = bacc.Bacc("TRN2", target_bir_lowering=False)
    xt = nc.dram_tensor("xt", [D_IN, NPC], mybir.dt.bfloat16, kind="ExternalInput")
    w = nc.dram_tensor("w", [D_IN, 3 * H], mybir.dt.bfloat16, kind="ExternalInput")
    qkvT = nc.dram_tensor(
        "qkvT", [3 * H, NPC], mybir.dt.bfloat16, kind="ExternalOutput"
    )

    ntiles = (NPC + K1_COLS - 1) // K1_COLS
    with tile.TileContext(nc) as tc:
        with ExitStack() as ctx:
            wp = ctx.enter_context(tc.tile_pool(name="wp", bufs=1))
            xp = ctx.enter_context(tc.tile_pool(name="xp", bufs=3))
            pp = ctx.enter_context(tc.tile_pool(name="pp", bufs=4, space="PSUM"))
            op = ctx.enter_context(tc.tile_pool(name="op", bufs=3))
            w0 = wp.tile([P, 3 * H], mybir.dt.bfloat16, tag="w0")
            w1 = wp.tile([P, 3 * H], mybir.dt.bfloat16, tag="w1")
            nc.sync.dma_start(w0[:], w[0:P, :])
            nc.sync.dma_start(w1[:], w[P : 2 * P, :])
            for t in range(ntiles):
                c0 = t * K1_COLS
                m = min(K1_COLS, NPC - c0)
                x0 = xp.tile([P, K1_COLS], mybir.dt.bfloat16, tag="x0")
                x1 = xp.tile([P, K1_COLS], mybir.dt.bfloat16, tag="x1")
                nc.sync.dma_start(x0[:, :m], xt[0:P, c0 : c0 + m])
                nc.sync.dma_start(x1[:, :m], xt[P : 2 * P, c0 : c0 + m])
                ot = op.tile([3 * H, K1_COLS], mybir.dt.bfloat16, tag="ot")
                for q0 in range(0, m, QUART):
                    mq = min(QUART, m - q0)
                    ps = pp.tile([3 * H, QUART], mybir.dt.float32, tag="ps")
                    nc.tensor.matmul(
                        ps[:, :mq], w0[:], x0[:, q0 : q0 + mq], start=True, stop=False
                    )
                    nc.tensor.matmul(
                        ps[:, :mq], w1[:], x1[:, q0 : q0 + mq], start=False, stop=True
                    )
                    nc.vector.tensor_copy(ot[:, q0 : q0 + mq], ps[:, :mq])
                nc.sync.dma_start(qkvT[:, c0 : c0 + m], ot[:, :m])
    nc.compile()
    return nc


# ================================================================ kernel 2
def _build_k2(nt, tail):
    S = (nt - 1) * TI + tail
    nc = bacc.Bacc("TRN2", target_bir_lowering=False)
    kvt = nc.dram_tensor("kvt", [P, CHUNK, 4], mybir.dt.bfloat16, kind="ExternalInput")
    idx = nc.dram_tensor("idx", [P, S // 16], mybir.dt.int16, kind="ExternalInput")
    qexp = nc.dram_tensor("qexp", [P, S, 2], mybir.dt.bfloat16, kind="ExternalInput")
    exd = nc.dram_tensor("exd", [nt, P, TI], mybir.dt.float32, kind="ExternalOutput")
    exvd = nc.dram_tensor(
        "exvd", [nt, P, TI, 2], mybir.dt.bfloat16, kind="ExternalOutput"
    )
    onesd = nc.dram_tensor("onesd", [P, P], mybir.dt.float32, kind="ExternalInput")

    with tile.TileContext(nc) as tc:
        with ExitStack() as ctx:
            tp = ctx.enter_context(tc.tile_pool(name="tp", bufs=1))
            gp = ctx.enter_context(tc.tile_pool(name="gp", bufs=2))
            qp = ctx.enter_context(tc.tile_pool(name="qp", bufs=3))
            prp = ctx.enter_context(tc.tile_pool(name="prp", bufs=2))
            srp = ctx.enter_context(tc.tile_pool(name="srp", bufs=1))
            psp = ctx.enter_context(tc.tile_pool(name="psp", bufs=2, space="PSUM"))
            exp_ = ctx.enter_context(tc.tile_pool(name="exp", bufs=1))
            evp = ctx.enter_context(tc.tile_pool(name="evp", bufs=2))

            tt = tp.tile([P, CHUNK, 4], mybir.dt.bfloat16, tag="tt")
            nc.sync.dma_start(tt[:], kvt[:, :, :])
            itall = tp.tile([P, S // 16], mybir.dt.int16, tag="itall")
            nc.sync.dma_start(itall[:], idx[:, :])
            # block-diagonal ones [128,128]: 16x16 ones blocks on the diagonal
            # -> one matmul sums each chunk's 16 word-partitions, replicated.
            ones = tp.tile([P, P], mybir.dt.float32, tag="ones")
            nc.sync.dma_start(ones[:], onesd[:, :])

            # software-pipelined input DMAs: issue it/qe for call i+1 before
            # call i's compute chain so the sync queue never blocks them
            # behind the exd/exvd output DMAs (which wait on DVE results).
            widths = [TI] * (nt - 1) + [tail]
            offs = [sum(widths[:j]) for j in range(nt)]
            qes = [None] * nt

            def stage_inputs(j):
                nj, cj = widths[j], offs[j]
                qej = qp.tile([P, TI, 2], mybir.dt.bfloat16, tag="qe")
                nc.sync.dma_start(qej[:, :nj, :], qexp[:, cj : cj + nj, :])
                qes[j] = qej

            gs = [None] * nt

            def compute_chain(j):
                """Full DVE/PE/ACT/out chain for call j (inputs already done)."""
                nj = widths[j]
                g, qe = gs[j], qes[j]
                prod = prp.tile([P, TI, 2], mybir.dt.bfloat16, tag="prod")
                nc.vector.tensor_tensor(
                    out=prod[:, :nj, :], in0=qe[:, :nj, :], in1=g[:, :nj, 0:2],
                    op=mybir.AluOpType.mult,
                )
                pr = srp.tile([P, TI], mybir.dt.float32, tag="pr")
                nc.vector.tensor_reduce(
                    out=pr[:, :nj], in_=prod[:, :nj, :], axis=mybir.AxisListType.X,
                    op=mybir.AluOpType.add,
                )
                ps = psp.tile([P, TI], mybir.dt.float32, tag="ps")
                for q in range((nj + QUART - 1) // QUART):
                    qn = min(QUART, nj - q * QUART)
                    nc.tensor.matmul(
                        ps[:, q * QUART : q * QUART + qn],
                        ones[:],
                        pr[:, q * QUART : q * QUART + qn],
                        start=True, stop=True,
                    )
                ex = exp_.tile([P, TI], mybir.dt.float32, tag="ex")
                nc.scalar.activation(
                    ex[:, :nj], ps[:, :nj], mybir.ActivationFunctionType.Exp,
                    scale=1.0 / DK,
                )
                ev = evp.tile([P, TI, 2], mybir.dt.bfloat16, tag="ev")
                nc.vector.tensor_tensor(
                    out=ev[:, :nj, :],
                    in0=ex[:, :nj].rearrange("p (n o) -> p n o", o=1).to_broadcast(
                        [P, nj, 2]
                    ),
                    in1=g[:, :nj, 2:4],
                    op=mybir.AluOpType.mult,
                )
                nc.scalar.dma_start(exd[j, :, :nj], ex[:, :nj])
                nc.scalar.dma_start(exvd[j, :, :nj, :], ev[:, :nj, :])

            # Compute chain for call i-1 is issued AFTER gather(i): the tile
            # framework's conservative queue-progress waits make each gather
            # wait for all previously-issued DVE work, and call i's chain can
            # only start once gather(i) finishes. Deferring the chain one call
            # keeps every DVE op issued between gathers immediately runnable,
            # so it overlaps the in-flight gather instead of stalling the next.
            stage_inputs(0)
            for i in range(nt):
                ni = widths[i]
                g = gp.tile([P, TI, 4], mybir.dt.bfloat16, tag="g")
                gs[i] = g
                nc.gpsimd.ap_gather(
                    out_ap=g[:, :ni, :],
                    in_ap=tt[:],
                    idxs_ap=itall[:, offs[i] // 16 : (offs[i] + ni) // 16],
                    channels=P,
                    num_elems=CHUNK,
                    d=4,
                    num_idxs=ni,
                )
                if i + 1 < nt:
                    stage_inputs(i + 1)
                if i >= 1:
                    compute_chain(i - 1)
            compute_chain(nt - 1)
    nc.compile()
    return nc


# ================================================================ driver
def kernel(X, edge_index, Wq, Wk, Wv):
    X = np.ascontiguousarray(np.asarray(X, dtype=np.float32))
    Wq = np.asarray(Wq, dtype=np.float32)
    Wk = np.asarray(Wk, dtype=np.float32)
    Wv = np.asarray(Wv, dtype=np.float32)
    ei = np.asarray(edge_index)

    cores, node_maps, nt, tail = _prep(ei)

    # ---- kernel 1: projections
    if "k1" not in _cache:
        _cache["k1"] = _build_k1()
    k1 = _cache["k1"]
    w_cat = np.concatenate([Wq, Wk, Wv], axis=1).astype(BF16)  # [256, 96]
    in1 = [
        {"xt": np.ascontiguousarray(X[c * NPC : (c + 1) * NPC].T).astype(BF16),
         "w": w_cat}
        for c in range(NCORES)
    ]
    r1 = run_bass_kernel_spmd(k1, in1, core_ids=list(range(NCORES)))
    LAST_TIMES["k1"] = r1.exec_time_ns
    qkv = [np.ascontiguousarray(r1.results[c]["qkvT"].T) for c in range(NCORES)]

    # K|V table, packed bf16 words per core (each core has its own balanced
    # node->chunk map): kvt[16k+c, l, :] = [K[g,2c], K[g,2c+1], V[g,2c],
    # V[g,2c+1]] with g = node_of[k, l].
    Kg = np.concatenate([q[:, H : 2 * H] for q in qkv], axis=0).astype(BF16)
    Vg = np.concatenate([q[:, 2 * H : 3 * H] for q in qkv], axis=0).astype(BF16)
    kvts = []
    for c in range(NCORES):
        kvt = np.zeros((P, CHUNK, 4), dtype=BF16)
        for k in range(NCHUNK):
            nodes = node_maps[c][k]
            kw = Kg[nodes].reshape(CHUNK, 16, 2).transpose(1, 0, 2)
            vw = Vg[nodes].reshape(CHUNK, 16, 2).transpose(1, 0, 2)
            kvt[16 * k : 16 * k + 16, :, 0:2] = kw
            kvt[16 * k : 16 * k + 16, :, 2:4] = vw
        kvts.append(kvt)

    # ---- kernel 2
    if ("k2", nt, tail) not in _cache:
        _cache[("k2", nt, tail)] = _build_k2(nt, tail)
    k2 = _cache[("k2", nt, tail)]
    onesd = np.kron(np.eye(NCHUNK, dtype=np.float32), np.ones((16, 16), np.float32))
    in2 = []
    for c in range(NCORES):
        Qb = qkv[c][:, :H].astype(BF16)
        m = _pack_core_inputs(cores[c], nt, tail, Qb, kvts[c])
        m["onesd"] = onesd
        in2.append(m)
    r2 = run_bass_kernel_spmd(k2, in2, core_ids=list(range(NCORES)))
    LAST_TIMES["k2"] = r2.exec_time_ns

    # ---- host combine
    out = np.empty((N, H), dtype=np.float32)
    for c in range(NCORES):
        out[c * NPC : (c + 1) * NPC] = _combine_core(
            cores[c], r2.results[c]["exd"], r2.results[c]["exvd"], nt, tail
        )
    return out


# revision 25
# speedup vs baseline: 1.0004x; 1.0004x over previous
"""Trainium2 Bass kernel for nn_MemoryAggregator — ap_gather edition.

Reference computation:
    Q = X@Wq; K = X@Wk; V = X@Wv            (X [100000,256], W [256,32])
    scores_e = <Q[src_e], K[dst_e]> / sqrt(32)   over 1.6M edges
    out[n]   = softmax-weighted sum over n's edges of V[dst_e]   ([100000,32])

Strategy (8 NeuronCores, SPMD, edges sharded by src):
  kernel1: per-core QKV projections (PE matmul), as before.
  kernel2: per core, per-edge pipeline driven by GPSIMD ap_gather:
    - K|V table [128 chan, 12500, 4] bf16 resident in SBUF: channel 16k+c
      (dst-chunk k of 12500 nodes, word c) holds [K[2c],K[2c+1],V[2c],V[2c+1]]
      of each chunk node. Each of the 8 GPSIMD CPUs gathers its own chunk's
      edge stream (d=4, ~26.5 ns/idx fixed cost, streams run in parallel).
    - DVE: prod = Qexp * Kgathered (bf16), pair-reduce -> [128, TI] f32.
    - PE: per (chunk, 512-col quarter) ones[16,16]-matmul reduces the 16
      word-partitions -> scores replicated across the chunk's 16 partitions.
    - ACT: exp(score/sqrt(32)) (no max subtraction; scores are O(10), safe).
    - DVE: exv = ex * Vgathered (bf16) -> per-edge partials to HBM.
  host:    segment sums per src (bincount) + divide, as in the baseline.

Softmax max-subtraction is dropped: scores ~ N(0,4), exp safe in f32.
"""
import math
from contextlib import ExitStack

import numpy as np
import ml_dtypes

import concourse.bass as bass
import concourse.tile as tile
from concourse import bacc, mybir
from concourse.bass_utils import run_bass_kernel_spmd

# ---------------------------------------------------------------- dimensions
N = 100000
E = 1600000
D_IN = 256
H = 32
DK = math.sqrt(H)
NCORES = 8
NPC = N // NCORES          # 12500 nodes per core (src shard)
NCHUNK = 8                 # dst chunks, one per GPSIMD CPU group
CHUNK = N // NCHUNK        # 12500
P = 128
TI = 2048                  # edges per chunk-stream per ap_gather call
QUART = 512                # PSUM bank col width (f32)

BF16 = ml_dtypes.bfloat16

_cache = {}
LAST_TIMES = {}


# ================================================================ host prep
def _prep(edge_index):
    """Per-core, per-chunk edge streams (sorted by src within chunk).

    Nodes are assigned to the 8 dst-chunks per core by degree-aware snake
    packing so the 8 gather streams (whose max sets the gather count) are
    balanced to within a few edges instead of the ~±3 sigma of a fixed
    dst//12500 split. node_of[k, l] = global node at (chunk k, local l)."""
    src = np.asarray(edge_index[0], dtype=np.int64)
    dst = np.asarray(edge_index[1], dtype=np.int64)
    core = src // NPC
    cores = []
    node_maps = []
    max_len = 0
    rows = np.arange(N) // NCHUNK
    colp = np.arange(N) % NCHUNK
    snake_chunk = np.where(rows % 2 == 0, colp, NCHUNK - 1 - colp)
    for c in range(NCORES):
        m = core == c
        s_l = src[m] - c * NPC
        d = dst[m]
        cnt = np.bincount(d, minlength=N)
        order = np.argsort(-cnt, kind="stable")
        chunk_of = np.empty(N, dtype=np.int64)
        local_of = np.empty(N, dtype=np.int64)
        chunk_of[order] = snake_chunk
        local_of[order] = rows
        node_of = np.empty((NCHUNK, CHUNK), dtype=np.int64)
        node_of[snake_chunk, rows] = order
        chunk = chunk_of[d]
        dl_all = local_of[d]
        o = np.lexsort((s_l, chunk))
        s_l, dl_all, chunk = s_l[o], dl_all[o], chunk[o]
        bounds = np.searchsorted(chunk, np.arange(NCHUNK + 1))
        streams = []
        for k in range(NCHUNK):
            lo, hi = bounds[k], bounds[k + 1]
            streams.append((s_l[lo:hi], dl_all[lo:hi]))
            max_len = max(max_len, hi - lo)
        cores.append(streams)
        node_maps.append(node_of)
    nt = (max_len + TI - 1) // TI
    tail = max_len - (nt - 1) * TI
    tail = ((tail + 15) // 16) * 16  # num_idxs multiple of 16
    return cores, node_maps, nt, tail


def _pack_core_inputs(streams, nt, tail, Qb, kvt):
    """Build idx / qexp tensors for one core."""
    S = (nt - 1) * TI + tail
    idx = np.zeros((P, S // 16), dtype=np.int16)
    qexp = np.zeros((P, S, 2), dtype=BF16)
    for k in range(NCHUNK):
        sl, dl = streams[k]
        L = len(sl)
        idx_k = np.zeros(S, dtype=np.int16)
        idx_k[:L] = dl.astype(np.int16)
        idx[16 * k : 16 * k + 16, :] = idx_k.reshape(-1, 16).T
        # qexp[16k+c, j, h] = Q[src_j, 2c+h]
        qb = Qb[sl]                      # [L, 32] bf16
        qexp[16 * k : 16 * k + 16, :L, :] = (
            qb.reshape(L, 16, 2).transpose(1, 0, 2)
        )
    return {"kvt": kvt, "idx": idx, "qexp": qexp}


def _combine_core(streams, exd, exvd, nt, tail):
    """Host segment sums + divide for one core. exd [nt,128,TI] f32,
    exvd [nt,128,TI,2] bf16 (last call only :tail valid)."""
    num = np.zeros((NPC, H), dtype=np.float64)
    den = np.zeros(NPC, dtype=np.float64)
    widths = [TI] * (nt - 1) + [tail]
    ex_flat = np.concatenate(
        [exd[i, :, : widths[i]] for i in range(nt)], axis=1
    )                                                          # [128, S]
    exv_flat = np.concatenate(
        [exvd[i, :, : widths[i], :].astype(np.float32) for i in range(nt)], axis=1
    )
    for k in range(NCHUNK):
        sl, _ = streams[k]
        L = len(sl)
        if L == 0:
            continue
        ex_k = ex_flat[16 * k, :L].astype(np.float64)          # [L]
        den += np.bincount(sl, weights=ex_k, minlength=NPC)
        # feats: exv_flat[16k+c, j, h] = ex*V[2c+h]
        blk = exv_flat[16 * k : 16 * k + 16, :L, :]            # [16, L, 2]
        feats = blk.transpose(1, 0, 2).reshape(L, H)           # [L, 32]
        for f in range(H):
            num[:, f] += np.bincount(sl, weights=feats[:, f], minlength=NPC)
    den = np.where(den == 0, 1.0, den)
    return (num / den[:, None]).astype(np.float32)


# ================================================================ kernel 1
K1_COLS = 2048


def _build_k1():
    """Weights-stationary bf16: out qkvT[96, NPC] = W.T @ X.T. Large column
    tiles keep the DMA-instruction count low (issue cost ~0.6 us each);
    matmuls split per 512 cols (one PSUM bank)."""
    nc = bacc.Bacc("TRN2", target_bir_lowering=False)
    xt = nc.dram_tensor("xt", [D_IN, NPC], mybir.dt.bfloat16, kind="ExternalInput")
    w = nc.dram_tensor("w", [D_IN, 3 * H], mybir.dt.bfloat16, kind="ExternalInput")
    qkvT = nc.dram_tensor(
        "qkvT", [3 * H, NPC], mybir.dt.bfloat16, kind="ExternalOutput"
    )

    ntiles = (NPC + K1_COLS - 1) // K1_COLS
    with tile.TileContext(nc) as tc:
        with ExitStack() as ctx:
            wp = ctx.enter_context(tc.tile_pool(name="wp", bufs=1))
            xp = ctx.enter_context(tc.tile_pool(name="xp", bufs=3))
            pp = ctx.enter_context(tc.tile_pool(name="pp", bufs=4, space="PSUM"))
            op = ctx.enter_context(tc.tile_pool(name="op", bufs=3))
            w0 = wp.tile([P, 3 * H], mybir.dt.bfloat16, tag="w0")
            w1 = wp.tile([P, 3 * H], mybir.dt.bfloat16, tag="w1")
            nc.sync.dma_start(w0[:], w[0:P, :])
            nc.sync.dma_start(w1[:], w[P : 2 * P, :])
            for t in range(ntiles):
                c0 = t * K1_COLS
                m = min(K1_COLS, NPC - c0)
                # one DMA per block: partition p gets rows (p, 128+p) of X
                xx = xp.tile([P, 2, K1_COLS], mybir.dt.bfloat16, tag="xx")
                nc.sync.dma_start(
                    xx[:, :, :m],
                    xt[:, c0 : c0 + m].rearrange("(a p) n -> p a n", a=2),
                )
                ot = op.tile([3 * H, K1_COLS], mybir.dt.bfloat16, tag="ot")
                for q0 in range(0, m, QUART):
                    mq = min(QUART, m - q0)
                    ps = pp.tile([3 * H, QUART], mybir.dt.float32, tag="ps")
                    nc.tensor.matmul(
                        ps[:, :mq], w0[:], xx[:, 0, q0 : q0 + mq], start=True, stop=False
                    )
                    nc.tensor.matmul(
                        ps[:, :mq], w1[:], xx[:, 1, q0 : q0 + mq], start=False, stop=True
                    )
                    nc.vector.tensor_copy(ot[:, q0 : q0 + mq], ps[:, :mq])
                nc.sync.dma_start(qkvT[:, c0 : c0 + m], ot[:, :m])
    nc.compile()
    return nc


# ================================================================ kernel 2
def _build_k2(nt, tail):
    S = (nt - 1) * TI + tail
    nc = bacc.Bacc("TRN2", target_bir_lowering=False)
    kvt = nc.dram_tensor("kvt", [P, CHUNK, 4], mybir.dt.bfloat16, kind="ExternalInput")
    idx = nc.dram_tensor("idx", [P, S // 16], mybir.dt.int16, kind="ExternalInput")
    qexp = nc.dram_tensor("qexp", [P, S, 2], mybir.dt.bfloat16, kind="ExternalInput")
    exd = nc.dram_tensor("exd", [nt, P, TI], mybir.dt.float32, kind="ExternalOutput")
    exvd = nc.dram_tensor(
        "exvd", [nt, P, TI, 2], mybir.dt.bfloat16, kind="ExternalOutput"
    )
    onesd = nc.dram_tensor("onesd", [P, P], mybir.dt.float32, kind="ExternalInput")

    with tile.TileContext(nc) as tc:
        with ExitStack() as ctx:
            tp = ctx.enter_context(tc.tile_pool(name="tp", bufs=1))
            gp = ctx.enter_context(tc.tile_pool(name="gp", bufs=2))
            qp = ctx.enter_context(tc.tile_pool(name="qp", bufs=3))
            prp = ctx.enter_context(tc.tile_pool(name="prp", bufs=2))
            srp = ctx.enter_context(tc.tile_pool(name="srp", bufs=1))
            psp = ctx.enter_context(tc.tile_pool(name="psp", bufs=2, space="PSUM"))
            exp_ = ctx.enter_context(tc.tile_pool(name="exp", bufs=1))
            evp = ctx.enter_context(tc.tile_pool(name="evp", bufs=2))

            tt = tp.tile([P, CHUNK, 4], mybir.dt.bfloat16, tag="tt")
            nc.sync.dma_start(tt[:], kvt[:, :, :])
            itall = tp.tile([P, S // 16], mybir.dt.int16, tag="itall")
            nc.sync.dma_start(itall[:], idx[:, :])
            # block-diagonal ones [128,128]: 16x16 ones blocks on the diagonal
            # -> one matmul sums each chunk's 16 word-partitions, replicated.
            ones = tp.tile([P, P], mybir.dt.float32, tag="ones")
            nc.sync.dma_start(ones[:], onesd[:, :])

            # software-pipelined input DMAs: issue it/qe for call i+1 before
            # call i's compute chain so the sync queue never blocks them
            # behind the exd/exvd output DMAs (which wait on DVE results).
            widths = [TI] * (nt - 1) + [tail]
            offs = [sum(widths[:j]) for j in range(nt)]
            qes = [None] * nt

            def stage_inputs(j):
                nj, cj = widths[j], offs[j]
                qej = qp.tile([P, TI, 2], mybir.dt.bfloat16, tag="qe")
                nc.sync.dma_start(qej[:, :nj, :], qexp[:, cj : cj + nj, :])
                qes[j] = qej

            gs = [None] * nt

            def compute_chain(j):
                """Full DVE/PE/ACT/out chain for call j (inputs already done)."""
                nj = widths[j]
                g, qe = gs[j], qes[j]
                prod = prp.tile([P, TI, 2], mybir.dt.bfloat16, tag="prod")
                nc.vector.tensor_tensor(
                    out=prod[:, :nj, :], in0=qe[:, :nj, :], in1=g[:, :nj, 0:2],
                    op=mybir.AluOpType.mult,
                )
                pr = srp.tile([P, TI], mybir.dt.float32, tag="pr")
                nc.vector.tensor_reduce(
                    out=pr[:, :nj], in_=prod[:, :nj, :], axis=mybir.AxisListType.X,
                    op=mybir.AluOpType.add,
                )
                ps = psp.tile([P, TI], mybir.dt.float32, tag="ps")
                for q in range((nj + QUART - 1) // QUART):
                    qn = min(QUART, nj - q * QUART)
                    nc.tensor.matmul(
                        ps[:, q * QUART : q * QUART + qn],
                        ones[:],
                        pr[:, q * QUART : q * QUART + qn],
                        start=True, stop=True,
                    )
                ex = exp_.tile([P, TI], mybir.dt.float32, tag="ex")
                nc.scalar.activation(
                    ex[:, :nj], ps[:, :nj], mybir.ActivationFunctionType.Exp,
                    scale=1.0 / DK,
                )
                ev = evp.tile([P, TI, 2], mybir.dt.bfloat16, tag="ev")
                nc.vector.tensor_tensor(
                    out=ev[:, :nj, :],
                    in0=ex[:, :nj].rearrange("p (n o) -> p n o", o=1).to_broadcast(
                        [P, nj, 2]
                    ),
                    in1=g[:, :nj, 2:4],
                    op=mybir.AluOpType.mult,
                )
                nc.scalar.dma_start(exd[j, :, :nj], ex[:, :nj])
                nc.scalar.dma_start(exvd[j, :, :nj, :], ev[:, :nj, :])

            # Compute chain for call i-1 is issued AFTER gather(i): the tile
            # framework's conservative queue-progress waits make each gather
            # wait for all previously-issued DVE work, and call i's chain can
            # only start once gather(i) finishes. Deferring the chain one call
            # keeps every DVE op issued between gathers immediately runnable,
            # so it overlaps the in-flight gather instead of stalling the next.
            stage_inputs(0)
            for i in range(nt):
                ni = widths[i]
                g = gp.tile([P, TI, 4], mybir.dt.bfloat16, tag="g")
                gs[i] = g
                nc.gpsimd.ap_gather(
                    out_ap=g[:, :ni, :],
                    in_ap=tt[:],
                    idxs_ap=itall[:, offs[i] // 16 : (offs[i] + ni) // 16],
                    channels=P,
                    num_elems=CHUNK,
                    d=4,
                    num_idxs=ni,
                )
                if i + 1 < nt:
                    stage_inputs(i + 1)
                if i >= 1:
                    compute_chain(i - 1)
            compute_chain(nt - 1)
    nc.compile()
    return nc


# ================================================================ driver
def kernel(X, edge_index, Wq, Wk, Wv):
    X = np.ascontiguousarray(np.asarray(X, dtype=np.float32))
    Wq = np.asarray(Wq, dtype=np.float32)
    Wk = np.asarray(Wk, dtype=np.float32)
    Wv = np.asarray(Wv, dtype=np.float32)
    ei = np.asarray(edge_index)

    cores, node_maps, nt, tail = _prep(ei)

    # ---- kernel 1: projections
    if "k1" not in _cache:
        _cache["k1"] = _build_k1()
    k1 = _cache["k1"]
    w_cat = np.concatenate([Wq, Wk, Wv], axis=1).astype(BF16)  # [256, 96]
    in1 = [
        {"xt": np.ascontiguousarray(X[c * NPC : (c + 1) * NPC].T).astype(BF16),
         "w": w_cat}
        for c in range(NCORES)
    ]
    r1 = run_bass_kernel_spmd(k1, in1, core_ids=list(range(NCORES)))
    LAST_TIMES["k1"] = r1.exec_time_ns
    qkv = [np.ascontiguousarray(r1.results[c]["qkvT"].T) for c in range(NCORES)]

    # K|V table, packed bf16 words per core (each core has its own balanced
    # node->chunk map): kvt[16k+c, l, :] = [K[g,2c], K[g,2c+1], V[g,2c],
    # V[g,2c+1]] with g = node_of[k, l].
    Kg = np.concatenate([q[:, H : 2 * H] for q in qkv], axis=0).astype(BF16)
    Vg = np.concatenate([q[:, 2 * H : 3 * H] for q in qkv], axis=0).astype(BF16)
    kvts = []
    for c in range(NCORES):
        kvt = np.zeros((P, CHUNK, 4), dtype=BF16)
        for k in range(NCHUNK):
            nodes = node_maps[c][k]
            kw = Kg[nodes].reshape(CHUNK, 16, 2).transpose(1, 0, 2)
            vw = Vg[nodes].reshape(CHUNK, 16, 2).transpose(1, 0, 2)
            kvt[16 * k : 16 * k + 16, :, 0:2] = kw
            kvt[16 * k : 16 * k + 16, :, 2:4] = vw
        kvts.append(kvt)

    # ---- kernel 2
    if ("k2", nt, tail) not in _cache:
        _cache[("k2", nt, tail)] = _build_k2(nt, tail)
    k2 = _cache[("k2", nt, tail)]
    onesd = np.kron(np.eye(NCHUNK, dtype=np.float32), np.ones((16, 16), np.float32))
    in2 = []
    for c in range(NCORES):
        Qb = qkv[c][:, :H].astype(BF16)
        m = _pack_core_inputs(cores[c], nt, tail, Qb, kvts[c])
        m["onesd"] = onesd
        in2.append(m)
    r2 = run_bass_kernel_spmd(k2, in2, core_ids=list(range(NCORES)))
    LAST_TIMES["k2"] = r2.exec_time_ns

    # ---- host combine
    out = np.empty((N, H), dtype=np.float32)
    for c in range(NCORES):
        out[c * NPC : (c + 1) * NPC] = _combine_core(
            cores[c], r2.results[c]["exd"], r2.results[c]["exvd"], nt, tail
        )
    return out


# revision 26
# speedup vs baseline: 1.0079x; 1.0074x over previous
"""Trainium2 Bass kernel for nn_MemoryAggregator — ap_gather edition.

Reference computation:
    Q = X@Wq; K = X@Wk; V = X@Wv            (X [100000,256], W [256,32])
    scores_e = <Q[src_e], K[dst_e]> / sqrt(32)   over 1.6M edges
    out[n]   = softmax-weighted sum over n's edges of V[dst_e]   ([100000,32])

Strategy (8 NeuronCores, SPMD, edges sharded by src):
  kernel1: per-core QKV projections (PE matmul), as before.
  kernel2: per core, per-edge pipeline driven by GPSIMD ap_gather:
    - K|V table [128 chan, 12500, 4] bf16 resident in SBUF: channel 16k+c
      (dst-chunk k of 12500 nodes, word c) holds [K[2c],K[2c+1],V[2c],V[2c+1]]
      of each chunk node. Each of the 8 GPSIMD CPUs gathers its own chunk's
      edge stream (d=4, ~26.5 ns/idx fixed cost, streams run in parallel).
    - DVE: prod = Qexp * Kgathered (bf16), pair-reduce -> [128, TI] f32.
    - PE: per (chunk, 512-col quarter) ones[16,16]-matmul reduces the 16
      word-partitions -> scores replicated across the chunk's 16 partitions.
    - ACT: exp(score/sqrt(32)) (no max subtraction; scores are O(10), safe).
    - DVE: exv = ex * Vgathered (bf16) -> per-edge partials to HBM.
  host:    segment sums per src (bincount) + divide, as in the baseline.

Softmax max-subtraction is dropped: scores ~ N(0,4), exp safe in f32.
"""
import math
from contextlib import ExitStack

import numpy as np
import ml_dtypes

import concourse.bass as bass
import concourse.tile as tile
from concourse import bacc, mybir
from concourse.bass_utils import run_bass_kernel_spmd

# ---------------------------------------------------------------- dimensions
N = 100000
E = 1600000
D_IN = 256
H = 32
DK = math.sqrt(H)
NCORES = 8
NPC = N // NCORES          # 12500 nodes per core (src shard)
NCHUNK = 8                 # dst chunks, one per GPSIMD CPU group
CHUNK = N // NCHUNK        # 12500
P = 128
TI = 2048                  # edges per chunk-stream per ap_gather call
QUART = 512                # PSUM bank col width (f32)

BF16 = ml_dtypes.bfloat16

_cache = {}
LAST_TIMES = {}


# ================================================================ host prep
def _prep(edge_index):
    """Per-core, per-chunk edge streams (sorted by src within chunk).

    Nodes are assigned to the 8 dst-chunks per core by degree-aware snake
    packing so the 8 gather streams (whose max sets the gather count) are
    balanced to within a few edges instead of the ~±3 sigma of a fixed
    dst//12500 split. node_of[k, l] = global node at (chunk k, local l)."""
    src = np.asarray(edge_index[0], dtype=np.int64)
    dst = np.asarray(edge_index[1], dtype=np.int64)
    core = src // NPC
    cores = []
    node_maps = []
    max_len = 0
    rows = np.arange(N) // NCHUNK
    colp = np.arange(N) % NCHUNK
    snake_chunk = np.where(rows % 2 == 0, colp, NCHUNK - 1 - colp)
    for c in range(NCORES):
        m = core == c
        s_l = src[m] - c * NPC
        d = dst[m]
        cnt = np.bincount(d, minlength=N)
        order = np.argsort(-cnt, kind="stable")
        chunk_of = np.empty(N, dtype=np.int64)
        local_of = np.empty(N, dtype=np.int64)
        chunk_of[order] = snake_chunk
        local_of[order] = rows
        node_of = np.empty((NCHUNK, CHUNK), dtype=np.int64)
        node_of[snake_chunk, rows] = order
        chunk = chunk_of[d]
        dl_all = local_of[d]
        o = np.lexsort((s_l, chunk))
        s_l, dl_all, chunk = s_l[o], dl_all[o], chunk[o]
        bounds = np.searchsorted(chunk, np.arange(NCHUNK + 1))
        streams = []
        for k in range(NCHUNK):
            lo, hi = bounds[k], bounds[k + 1]
            streams.append((s_l[lo:hi], dl_all[lo:hi]))
            max_len = max(max_len, hi - lo)
        cores.append(streams)
        node_maps.append(node_of)
    nt = (max_len + TI - 1) // TI
    tail = max_len - (nt - 1) * TI
    tail = ((tail + 15) // 16) * 16  # num_idxs multiple of 16
    return cores, node_maps, nt, tail


def _pack_core_inputs(streams, nt, tail, Qb, kvt):
    """Build idx / qexp tensors for one core."""
    S = (nt - 1) * TI + tail
    idx = np.zeros((P, S // 16), dtype=np.int16)
    qexp = np.zeros((P, S, 2), dtype=BF16)
    for k in range(NCHUNK):
        sl, dl = streams[k]
        L = len(sl)
        idx_k = np.zeros(S, dtype=np.int16)
        idx_k[:L] = dl.astype(np.int16)
        idx[16 * k : 16 * k + 16, :] = idx_k.reshape(-1, 16).T
        # qexp[16k+c, j, h] = Q[src_j, 2c+h]
        qb = Qb[sl]                      # [L, 32] bf16
        qexp[16 * k : 16 * k + 16, :L, :] = (
            qb.reshape(L, 16, 2).transpose(1, 0, 2)
        )
    return {"kvt": kvt, "idx": idx, "qexp": qexp}


def _combine_core(streams, exd, exvd, nt, tail):
    """Host segment sums + divide for one core. exd [nt,128,TI] f32,
    exvd [nt,128,TI,2] bf16 (last call only :tail valid)."""
    num = np.zeros((NPC, H), dtype=np.float64)
    den = np.zeros(NPC, dtype=np.float64)
    widths = [TI] * (nt - 1) + [tail]
    ex_flat = np.concatenate(
        [exd[i, :, : widths[i]] for i in range(nt)], axis=1
    )                                                          # [128, S]
    exv_flat = np.concatenate(
        [exvd[i, :, : widths[i], :].astype(np.float32) for i in range(nt)], axis=1
    )
    for k in range(NCHUNK):
        sl, _ = streams[k]
        L = len(sl)
        if L == 0:
            continue
        ex_k = ex_flat[16 * k, :L].astype(np.float64)          # [L]
        den += np.bincount(sl, weights=ex_k, minlength=NPC)
        # feats: exv_flat[16k+c, j, h] = ex*V[2c+h]
        blk = exv_flat[16 * k : 16 * k + 16, :L, :]            # [16, L, 2]
        feats = blk.transpose(1, 0, 2).reshape(L, H)           # [L, 32]
        for f in range(H):
            num[:, f] += np.bincount(sl, weights=feats[:, f], minlength=NPC)
    den = np.where(den == 0, 1.0, den)
    return (num / den[:, None]).astype(np.float32)


# ================================================================ kernel 1
K1_COLS = 2048


def _build_k1():
    """Weights-stationary bf16: out qkvT[96, NPC] = W.T @ X.T. Large column
    tiles keep the DMA-instruction count low (issue cost ~0.6 us each);
    matmuls split per 512 cols (one PSUM bank)."""
    nc = bacc.Bacc("TRN2", target_bir_lowering=False)
    xt = nc.dram_tensor("xt", [D_IN, NPC], mybir.dt.bfloat16, kind="ExternalInput")
    w = nc.dram_tensor("w", [D_IN, 3 * H], mybir.dt.bfloat16, kind="ExternalInput")
    qkvT = nc.dram_tensor(
        "qkvT", [3 * H, NPC], mybir.dt.bfloat16, kind="ExternalOutput"
    )

    ntiles = (NPC + K1_COLS - 1) // K1_COLS
    with tile.TileContext(nc) as tc:
        with ExitStack() as ctx:
            wp = ctx.enter_context(tc.tile_pool(name="wp", bufs=1))
            xp = ctx.enter_context(tc.tile_pool(name="xp", bufs=3))
            pp = ctx.enter_context(tc.tile_pool(name="pp", bufs=4, space="PSUM"))
            op = ctx.enter_context(tc.tile_pool(name="op", bufs=3))
            w0 = wp.tile([P, 3 * H], mybir.dt.bfloat16, tag="w0")
            w1 = wp.tile([P, 3 * H], mybir.dt.bfloat16, tag="w1")
            nc.sync.dma_start(w0[:], w[0:P, :])
            nc.sync.dma_start(w1[:], w[P : 2 * P, :])
            for t in range(ntiles):
                c0 = t * K1_COLS
                m = min(K1_COLS, NPC - c0)
                x0 = xp.tile([P, K1_COLS], mybir.dt.bfloat16, tag="x0")
                x1 = xp.tile([P, K1_COLS], mybir.dt.bfloat16, tag="x1")
                nc.sync.dma_start(x0[:, :m], xt[0:P, c0 : c0 + m])
                nc.sync.dma_start(x1[:, :m], xt[P : 2 * P, c0 : c0 + m])
                ot = op.tile([3 * H, K1_COLS], mybir.dt.bfloat16, tag="ot")
                for q0 in range(0, m, QUART):
                    mq = min(QUART, m - q0)
                    ps = pp.tile([3 * H, QUART], mybir.dt.float32, tag="ps")
                    nc.tensor.matmul(
                        ps[:, :mq], w0[:], x0[:, q0 : q0 + mq], start=True, stop=False
                    )
                    nc.tensor.matmul(
                        ps[:, :mq], w1[:], x1[:, q0 : q0 + mq], start=False, stop=True
                    )
                    nc.vector.tensor_copy(ot[:, q0 : q0 + mq], ps[:, :mq])
                nc.sync.dma_start(qkvT[:, c0 : c0 + m], ot[:, :m])
    nc.compile()
    return nc


# ================================================================ kernel 2
def _build_k2(nt, tail):
    S = (nt - 1) * TI + tail
    nc = bacc.Bacc("TRN2", target_bir_lowering=False)
    kvt = nc.dram_tensor("kvt", [P, CHUNK, 4], mybir.dt.bfloat16, kind="ExternalInput")
    idx = nc.dram_tensor("idx", [P, S // 16], mybir.dt.int16, kind="ExternalInput")
    qexp = nc.dram_tensor("qexp", [P, S, 2], mybir.dt.bfloat16, kind="ExternalInput")
    exd = nc.dram_tensor("exd", [nt, P, TI], mybir.dt.float32, kind="ExternalOutput")
    exvd = nc.dram_tensor(
        "exvd", [nt, P, TI, 2], mybir.dt.bfloat16, kind="ExternalOutput"
    )
    onesd = nc.dram_tensor("onesd", [P, P], mybir.dt.float32, kind="ExternalInput")

    with tile.TileContext(nc) as tc:
        with ExitStack() as ctx:
            tp = ctx.enter_context(tc.tile_pool(name="tp", bufs=1))
            gp = ctx.enter_context(tc.tile_pool(name="gp", bufs=2))
            qp = ctx.enter_context(tc.tile_pool(name="qp", bufs=3))
            prp = ctx.enter_context(tc.tile_pool(name="prp", bufs=2))
            srp = ctx.enter_context(tc.tile_pool(name="srp", bufs=1))
            psp = ctx.enter_context(tc.tile_pool(name="psp", bufs=2, space="PSUM"))
            exp_ = ctx.enter_context(tc.tile_pool(name="exp", bufs=1))
            evp = ctx.enter_context(tc.tile_pool(name="evp", bufs=2))

            tt = tp.tile([P, CHUNK, 4], mybir.dt.bfloat16, tag="tt")
            nc.sync.dma_start(tt[:], kvt[:, :, :])
            itall = tp.tile([P, S // 16], mybir.dt.int16, tag="itall")
            nc.sync.dma_start(itall[:], idx[:, :])
            # block-diagonal ones [128,128]: 16x16 ones blocks on the diagonal
            # -> one matmul sums each chunk's 16 word-partitions, replicated.
            ones = tp.tile([P, P], mybir.dt.float32, tag="ones")
            nc.sync.dma_start(ones[:], onesd[:, :])

            # software-pipelined input DMAs: issue it/qe for call i+1 before
            # call i's compute chain so the sync queue never blocks them
            # behind the exd/exvd output DMAs (which wait on DVE results).
            widths = [TI] * (nt - 1) + [tail]
            offs = [sum(widths[:j]) for j in range(nt)]
            qes = [None] * nt

            def stage_inputs(j):
                nj, cj = widths[j], offs[j]
                qej = qp.tile([P, TI, 2], mybir.dt.bfloat16, tag="qe")
                nc.sync.dma_start(qej[:, :nj, :], qexp[:, cj : cj + nj, :])
                qes[j] = qej

            gs = [None] * nt

            def compute_chain(j):
                """Full DVE/PE/ACT/out chain for call j (inputs already done)."""
                nj = widths[j]
                g, qe = gs[j], qes[j]
                prod = prp.tile([P, TI, 2], mybir.dt.bfloat16, tag="prod")
                nc.vector.tensor_tensor(
                    out=prod[:, :nj, :], in0=qe[:, :nj, :], in1=g[:, :nj, 0:2],
                    op=mybir.AluOpType.mult,
                )
                pr = srp.tile([P, TI], mybir.dt.float32, tag="pr")
                nc.vector.tensor_reduce(
                    out=pr[:, :nj], in_=prod[:, :nj, :], axis=mybir.AxisListType.X,
                    op=mybir.AluOpType.add,
                )
                ps = psp.tile([P, TI], mybir.dt.float32, tag="ps")
                for q in range((nj + QUART - 1) // QUART):
                    qn = min(QUART, nj - q * QUART)
                    nc.tensor.matmul(
                        ps[:, q * QUART : q * QUART + qn],
                        ones[:],
                        pr[:, q * QUART : q * QUART + qn],
                        start=True, stop=True,
                    )
                ex = exp_.tile([P, TI], mybir.dt.float32, tag="ex")
                nc.scalar.activation(
                    ex[:, :nj], ps[:, :nj], mybir.ActivationFunctionType.Exp,
                    scale=1.0 / DK,
                )
                ev = evp.tile([P, TI, 2], mybir.dt.bfloat16, tag="ev")
                nc.vector.tensor_tensor(
                    out=ev[:, :nj, :],
                    in0=ex[:, :nj].rearrange("p (n o) -> p n o", o=1).to_broadcast(
                        [P, nj, 2]
                    ),
                    in1=g[:, :nj, 2:4],
                    op=mybir.AluOpType.mult,
                )
                nc.scalar.dma_start(exd[j, :, :nj], ex[:, :nj])
                nc.scalar.dma_start(exvd[j, :, :nj, :], ev[:, :nj, :])

            # Compute chain for call i-1 is issued AFTER gather(i): the tile
            # framework's conservative queue-progress waits make each gather
            # wait for all previously-issued DVE work, and call i's chain can
            # only start once gather(i) finishes. Deferring the chain one call
            # keeps every DVE op issued between gathers immediately runnable,
            # so it overlaps the in-flight gather instead of stalling the next.
            stage_inputs(0)
            for i in range(nt):
                ni = widths[i]
                g = gp.tile([P, TI, 4], mybir.dt.bfloat16, tag="g")
                gs[i] = g
                nc.gpsimd.ap_gather(
                    out_ap=g[:, :ni, :],
                    in_ap=tt[:],
                    idxs_ap=itall[:, offs[i] // 16 : (offs[i] + ni) // 16],
                    channels=P,
                    num_elems=CHUNK,
                    d=4,
                    num_idxs=ni,
                )
                if i + 1 < nt:
                    stage_inputs(i + 1)
                if i >= 1:
                    compute_chain(i - 1)
            compute_chain(nt - 1)
    nc.compile()
    return nc


# ================================================================ driver
def kernel(X, edge_index, Wq, Wk, Wv):
    X = np.ascontiguousarray(np.asarray(X, dtype=np.float32))
    Wq = np.asarray(Wq, dtype=np.float32)
    Wk = np.asarray(Wk, dtype=np.float32)
    Wv = np.asarray(Wv, dtype=np.float32)
    ei = np.asarray(edge_index)

    cores, node_maps, nt, tail = _prep(ei)

    # ---- kernel 1: projections
    if "k1" not in _cache:
        _cache["k1"] = _build_k1()
    k1 = _cache["k1"]
    w_cat = np.concatenate([Wq, Wk, Wv], axis=1).astype(BF16)  # [256, 96]
    in1 = [
        {"xt": np.ascontiguousarray(X[c * NPC : (c + 1) * NPC].T).astype(BF16),
         "w": w_cat}
        for c in range(NCORES)
    ]
    r1 = run_bass_kernel_spmd(k1, in1, core_ids=list(range(NCORES)))
    LAST_TIMES["k1"] = r1.exec_time_ns
    qkv = [np.ascontiguousarray(r1.results[c]["qkvT"].T) for c in range(NCORES)]

    # K|V table, packed bf16 words per core (each core has its own balanced
    # node->chunk map): kvt[16k+c, l, :] = [K[g,2c], K[g,2c+1], V[g,2c],
    # V[g,2c+1]] with g = node_of[k, l].
    Kg = np.concatenate([q[:, H : 2 * H] for q in qkv], axis=0).astype(BF16)
    Vg = np.concatenate([q[:, 2 * H : 3 * H] for q in qkv], axis=0).astype(BF16)
    kvts = []
    for c in range(NCORES):
        kvt = np.zeros((P, CHUNK, 4), dtype=BF16)
        for k in range(NCHUNK):
            nodes = node_maps[c][k]
            kw = Kg[nodes].reshape(CHUNK, 16, 2).transpose(1, 0, 2)
            vw = Vg[nodes].reshape(CHUNK, 16, 2).transpose(1, 0, 2)
            kvt[16 * k : 16 * k + 16, :, 0:2] = kw
            kvt[16 * k : 16 * k + 16, :, 2:4] = vw
        kvts.append(kvt)

    # ---- kernel 2
    if ("k2", nt, tail) not in _cache:
        _cache[("k2", nt, tail)] = _build_k2(nt, tail)
    k2 = _cache[("k2", nt, tail)]
    onesd = np.kron(np.eye(NCHUNK, dtype=np.float32), np.ones((16, 16), np.float32))
    in2 = []
    for c in range(NCORES):
        Qb = qkv[c][:, :H].astype(BF16)
        m = _pack_core_inputs(cores[c], nt, tail, Qb, kvts[c])
        m["onesd"] = onesd
        in2.append(m)
    r2 = run_bass_kernel_spmd(k2, in2, core_ids=list(range(NCORES)))
    LAST_TIMES["k2"] = r2.exec_time_ns

    # ---- host combine
    out = np.empty((N, H), dtype=np.float32)
    for c in range(NCORES):
        out[c * NPC : (c + 1) * NPC] = _combine_core(
            cores[c], r2.results[c]["exd"], r2.results[c]["exvd"], nt, tail
        )
    return out
